# revision 6
# baseline (speedup 1.0000x reference)
"""GAT layer kernel for Trainium2 (8 NeuronCores, data-parallel over batch).

Reference computation (per graph b):
    Wh  = atoms @ W                      (N, FO)
    s1  = Wh @ a1 ; s2 = Wh @ a2         (N,)
    e   = leaky_relu(s1[:,None]+s2[None,:], 0.1)
    att = softmax(where(adj>0, e, -9e15), axis=1)
    out = elu(att @ Wh)

On-device formulation (no transcendental ever touches the NxN matrix):
    exp(leaky_relu(s)) = max(e^{s1_i} e^{s2_j}, e^{0.1 s1_i} e^{0.1 s2_j})
and because softmax row-normalizes, any per-row factor cancels, so with
r_i = min(e^{-0.9 s1_i}, 15000) (the clamp is row-uniform, hence exact):
    B_ij = max(v_j, r_i * q_j),  v = e^{s2-5}, q = e^{0.1 s2 - 5}
    att_ij = adj_ij B_ij / sum_j adj_ij B_ij
The 0/1 adjacency multiplies post-"exp" (exact: masked entries contribute 0
to numerator and denominator, equivalent to the reference's -9e15 trick).
The denominator comes free as a ones-column appended to Wh in the
P^T @ [Wh|1] matmul.

HBM-traffic-lean layout (the target_regime is memory):
  * adjacency ships host-pre-transposed (adj[g,j,i]=adjacency[g,i,j]),
    host-packed to fp8e4 (0/1 exact, 1/4 of the int32 bytes and 1/2 of an
    fp16 shipment), and host-rearranged partition-major [g, p, c, i] so the
    per-partition DRAM run is NCH*N contiguous bytes (128 fat descriptors).
    A single gpsimd (SWDGE) cast-DMA per graph upcasts fp8 -> fp16 in
    flight, so the DVE mask pass still runs on 2-byte operands at full
    DVE perf modes.
  * atoms ship host-transposed+fp16 [g, F_IN, N]: the on-device PE
    block-transposes and their PSUM staging disappear entirely.
  * scores are built in TRANSPOSED [j,i] form: one fused tensor_scalar
    (mult+max, two per-partition scalar ptrs q_j/v_j, split DVE/gpsimd)
    builds em rows max(q_j * r, v_j) against an r-broadcast tile during
    the adjacency DMA, then two merged all-SBUF fp16 tensor_tensor ops
    [P, 4, N] mask it into the P^T rows used as matmul lhsT.
Attention matmuls accumulate h rows for all 8 i-chunks across the jc
loop in two PSUM banks per graph, ic-major so each PSUM region's
accumulation group is contiguous (interleaved groups within a bank
lose all but the last-started group on HW).  The softmax divide folds
into DVE mults with a reciprocal broadcast.
"""

import numpy as np
from contextlib import ExitStack

import concourse.bass as bass
import concourse.tile as tile
import concourse.mybir as mybir
from concourse.masks import make_identity

dt = mybir.dt
Alu = mybir.AluOpType
Act = mybir.ActivationFunctionType

N = 1024          # nodes per graph
F_IN = 128        # input features
FO = 64           # output features
P = 128           # partitions
NCH = N // P      # 8 node chunks
N_CORES = 8
B_FULL = 64
M_SHIFT = 6.0     # exponent recentering for v = exp(s2 - 3)

# engine-split knobs (tuned via timeline sim)
STT_DVE = 8       # j-chunks fused on DVE; the rest Pool-em + DVE-mask
EM_DVE = 4        # of the 8 per-graph em rows, how many go to DVE (rest Pool)
EM1_POOL = True   # elu exp-1/min piece on gpsimd instead of DVE
HDIV_ACT = False  # softmax divide on ACT (8 scaled copies) vs DVE (2 TT)


def build_gat(bpc: int, reps: int = 1) -> bass.Bass:
    """Emit the bass program for one core processing `bpc` graphs."""
    nc = bass.Bass()
    atomsT = nc.declare_dram_parameter("atomsT", [bpc, F_IN, N], dt.float16,
                                       isOutput=False)
    # adjacency values are 0/1 — shipped host-repacked as fp8e4 (exact, 1/4
    # the int32 HBM traffic), host-PRE-TRANSPOSED (adj[g,j,i]=adjacency[g,i,j])
    # and partition-major so one SWDGE cast-DMA per graph lands fp16 in SBUF.
    adj = nc.declare_dram_parameter("adj", [bpc, P, NCH, N], dt.float8e4,
                                    isOutput=False)
    wext = nc.declare_dram_parameter("wext", [F_IN, FO + 2], dt.float16,
                                     isOutput=False)
    selmat = nc.declare_dram_parameter("selmat", [NCH, NCH * P], dt.float16,
                                       isOutput=False)
    out = nc.declare_dram_parameter("out", [bpc, N, FO], dt.float16, isOutput=True)

    with tile.TileContext(nc) as tc, ExitStack() as ctx:
        consts = ctx.enter_context(tc.tile_pool(name="consts", bufs=1))
        # PSUM budget (8 banks x 2KB): h 4 + t 2 + mm 2
        psum = ctx.enter_context(tc.tile_pool(name="psum", bufs=2, space="PSUM"))
        gbuf = ctx.enter_context(tc.tile_pool(name="gbuf", bufs=2))
        cbuf = ctx.enter_context(tc.tile_pool(name="cbuf", bufs=4))
        fbuf = ctx.enter_context(tc.tile_pool(name="fbuf", bufs=2))

        ident_b = consts.tile([P, P], dt.float16, tag="idb")
        make_identity(nc, ident_b)
        wext_sb = consts.tile([P, FO + 2], dt.float16, tag="wext")
        nc.sync.dma_start(out=wext_sb, in_=wext[:, :])
        bias_mh = consts.tile([P, 1], dt.float32, tag="bmh")
        nc.vector.memset(bias_mh, -M_SHIFT / 2)
        bias_z = consts.tile([P, 1], dt.float32, tag="bz")
        nc.vector.memset(bias_z, 0.0)
        # sel[:, c*P:(c+1)*P] is all-ones in row c: K=8 matmul with it as
        # stationary broadcasts row c of an [8, 128] tile to all partitions.
        sel_sb = consts.tile([NCH, NCH * P], dt.float16, tag="sel")
        nc.sync.dma_start(out=sel_sb, in_=selmat[:, :])

        def precompute(g):
            # ---------------- per-graph precompute (small) ----------------
            atT_sb = gbuf.tile([P, N], dt.float16, tag="atT", name=f"atT_{g}")
            nc.sync.dma_start(out=atT_sb, in_=atomsT[g])

            # [Wh | s1 | s2] = atoms_chunk @ [W | Wa1 | Wa2]; Wh and the
            # ones column get scaled by v_j = exp(s2_j - 3) (the column
            # factor of B'_ij = v_j * max(w_i, u_j)).
            whones = gbuf.tile([P, NCH, FO + 1], dt.float16, tag="whones",
                               name=f"whones_{g}")
            s12 = gbuf.tile([P, NCH, 2], dt.float32, tag="s12", name=f"s12_{g}")
            vcols = gbuf.tile([P, NCH], dt.float32, tag="vcols", name=f"vcols_{g}")
            for h in range(2):
                whc_ps = psum.tile([P, 4, FO + 2], dt.float32, tag="t",
                                   name=f"whc_ps_{g}_{h}")
                for cc in range(4):
                    c = h * 4 + cc
                    nc.tensor.matmul(whc_ps[:, cc, :],
                                     lhsT=atT_sb[:, c * P:(c + 1) * P],
                                     rhs=wext_sb, start=True, stop=True)
                nc.vector.tensor_copy(out=s12[:, h * 4:(h + 1) * 4, :],
                                      in_=whc_ps[:, :, FO:FO + 2])
                nc.scalar.activation(vcols[:, h * 4:(h + 1) * 4],
                                     s12[:, h * 4:(h + 1) * 4, 1], Act.Exp,
                                     bias=bias_mh, scale=1.0)
                for cc in range(4):
                    c = h * 4 + cc
                    nc.scalar.mul(whones[:, c, 0:FO], whc_ps[:, cc, 0:FO],
                                  vcols[:, c:c + 1])
            nc.vector.tensor_copy(out=whones[:, :, FO], in_=vcols)

            # w_i = min(exp(0.9 s1), 60000) broadcast; u_j = min(exp(-0.9 s2), 60000)
            rraw = gbuf.tile([P, NCH], dt.float32, tag="rraw", name=f"rraw_{g}")
            nc.scalar.activation(rraw, s12[:, :, 0], Act.Exp, bias=bias_z, scale=0.9)
            rcols16 = gbuf.tile([P, NCH], dt.float16, tag="rcols16",
                                name=f"rcols16_{g}")
            nc.vector.tensor_scalar(rcols16, rraw, 60000.0, None, Alu.min)
            uraw = gbuf.tile([P, NCH], dt.float32, tag="uraw", name=f"uraw_{g}")
            nc.scalar.activation(uraw, s12[:, :, 1], Act.Exp, bias=bias_z, scale=-0.9)
            qcols = gbuf.tile([P, NCH], dt.float32, tag="qcols", name=f"qcols_{g}")
            nc.vector.tensor_scalar(qcols, uraw, 60000.0, None, Alu.min)

            # broadcast w across partitions: rb[p, i] = w_i
            rT_ps = psum.tile([NCH, P], dt.float16, tag="t", name=f"rT_ps_{g}")
            nc.tensor.transpose(rT_ps, rcols16, ident_b)
            rT_sb = gbuf.tile([NCH, P], dt.float16, tag="rT", name=f"rT_{g}")
            nc.vector.tensor_copy(out=rT_sb, in_=rT_ps)
            rb_ps = psum.tile([P, N], dt.float32, tag="mm", bufs=1, name=f"rb_ps_{g}")
            for c in range(NCH):
                nc.tensor.matmul(rb_ps[:, c * P:(c + 1) * P],
                                 lhsT=sel_sb[:, c * P:(c + 1) * P],
                                 rhs=rT_sb, start=True, stop=True)
            rb = gbuf.tile([P, N], dt.float16, tag="rb", name=f"rb_{g}")
            nc.scalar.copy(out=rb, in_=rb_ps)

            adjf = [cbuf.tile([P, 4, N], dt.float16, tag="adjf", bufs=4,
                              name=f"adjf_{g}_{h}") for h in range(2)]
            h_ps = [psum.tile([P, 4, P], dt.float32, tag="h", bufs=4,
                              name=f"h_ps_{g}_{t}") for t in range(2)]
            res_g = gbuf.tile([P, NCH, FO], dt.float16, tag="res", name=f"res_{g}")
            return dict(adjf=adjf, whones=whones, vcols=vcols, qcols=qcols, rb=rb,
                        h_ps=h_ps, res=res_g, pms=[], ems=[])

        def adj_load(g, st):
            # SWDGE cast-DMAs: fp8 in DRAM -> fp16 in SBUF.  Two halves in
            # separate tiles so the first stt rows start after half the
            # transfer (partition-major host layout = fat descriptors).
            for h in range(2):
                nc.gpsimd.dma_start(out=st["adjf"][h], in_=adj[g][:, 4 * h:4 * h + 4, :])

        def emit_ems(g, st):
            # all j-chunks run as fused DVE scalar_tensor_tensor in pm_steps;
            # gpsimd instead owns the SWDGE cast-DMA and finalize pieces.
            if STT_DVE < NCH:
                em = cbuf.tile([P, NCH - STT_DVE, N], dt.float16, tag="em",
                               bufs=4, name=f"em_{g}")
                st["em"] = em
                for k, jc in enumerate(range(STT_DVE, NCH)):
                    nc.gpsimd.tensor_scalar(em[:, k, :], st["rb"],
                                            st["qcols"][:, jc:jc + 1], None,
                                            Alu.max)

        def pm_steps(g, st):
            # fused masked P^T rows: pm[j,:] = max(w, u_j) * adjT[j,:].
            # jc < STT_DVE: single DVE scalar_tensor_tensor; the rest mask
            # Pool-built em rows with a DVE tensor_tensor.
            for h in range(2):
                pm = cbuf.tile([P, 4, N], dt.float16, tag="pm", bufs=4,
                               name=f"pm_{g}_{h}")
                st["pms"].append(pm)
            for jc in range(STT_DVE):
                nc.vector.scalar_tensor_tensor(st["pms"][jc // 4][:, jc % 4, :],
                                               st["rb"], st["qcols"][:, jc:jc + 1],
                                               st["adjf"][jc // 4][:, jc % 4, :],
                                               Alu.max, Alu.mult)
            for k, jc in enumerate(range(STT_DVE, NCH)):
                nc.vector.tensor_tensor(st["pms"][jc // 4][:, jc % 4, :],
                                        st["em"][:, k, :],
                                        st["adjf"][:, jc, :], Alu.mult)

        def attn_mm(g, st, ic0=0, ic1=NCH):
            # h[i, 0:64] + denom col.  ic-major: each PSUM region's 8-matmul
            # accumulation group is contiguous — interleaved groups within a
            # bank lose all but the last-started group's accumulation on HW.
            for ic in range(ic0, ic1):
                for jc in range(NCH):
                    nc.tensor.matmul(st["h_ps"][ic // 4][:, ic % 4, 0:FO + 1],
                                     lhsT=st["pms"][jc // 4][:, jc % 4,
                                                            ic * P:(ic + 1) * P],
                                     rhs=st["whones"][:, jc, :],
                                     start=(jc == 0), stop=(jc == NCH - 1))

        def finalize(g, st):
            h_ps, res_g = st["h_ps"], st["res"]
            rec = fbuf.tile([P, NCH], dt.float32, tag="rec", name=f"rec_{g}")
            for t in range(2):
                nc.vector.reciprocal(rec[:, t * 4:(t + 1) * 4],
                                     h_ps[t][:, :, FO:FO + 1])
            hdiv = fbuf.tile([P, NCH, FO], dt.float32, tag="hdiv", name=f"hdiv_{g}")
            for c in range(NCH):
                eng = nc.gpsimd if HDIV_POOL else nc.scalar
                if eng is nc.scalar:
                    nc.scalar.mul(hdiv[:, c, :], h_ps[c // 4][:, c % 4, 0:FO],
                                  rec[:, c:c + 1])
                else:
                    eng.tensor_scalar(hdiv[:, c, :], h_ps[c // 4][:, c % 4, 0:FO],
                                      rec[:, c:c + 1], None, Alu.mult)
            hexp = fbuf.tile([P, NCH, FO], dt.float32, tag="hexp", name=f"hexp_{g}")
            nc.scalar.activation(hexp, hdiv, Act.Exp, bias=bias_z)
            em1 = fbuf.tile([P, NCH, FO], dt.float32, tag="em1", name=f"em1_{g}")
            eng = nc.gpsimd if EM1_POOL else nc.vector
            eng.tensor_scalar(em1, hexp, -1.0, 0.0, Alu.add, Alu.min)
            nc.vector.tensor_tensor(res_g, hdiv, em1, Alu.max)
            nc.scalar.dma_start(out=out[g].rearrange("(c p) f -> p c f", p=P),
                                in_=res_g)

        def start_graph(g):
            st = precompute(g)
            adj_load(g, st)
            return st

        for _rep in range(reps):
            states = {0: start_graph(0)}
            emit_ems(0, states[0])
            done = {}
            for g in range(bpc):
                if g + 1 < bpc:
                    states[g + 1] = start_graph(g + 1)
                st = states.pop(g)
                pm_steps(g, st)
                attn_mm(g, st)
                if g >= 1:
                    finalize(g - 1, done.pop(g - 1))
                done[g] = st
                if g + 1 < bpc:
                    emit_ems(g + 1, states[g + 1])
            finalize(bpc - 1, done.pop(bpc - 1))

    # HW allows at most one sync-wait per Matmult/Ldweights; Tile can emit
    # more.  Run the bacc lowering passes that move extra waits onto
    # ldweights / standalone event-semaphore instructions.
    import bass_rust as _br
    _br.move_matmul_waits_to_ldweights(nc.m)
    _br.generate_event_semaphores(nc)
    return nc


_NC_CACHE: dict[int, bass.Bass] = {}


def _get_nc(bpc: int) -> bass.Bass:
    if bpc not in _NC_CACHE:
        _NC_CACHE[bpc] = build_gat(bpc)
    return _NC_CACHE[bpc]


def _make_wext(W: np.ndarray, a: np.ndarray) -> np.ndarray:
    a1 = a[:FO, :]
    a2 = a[FO:, :]
    return np.concatenate([W, W @ a1, W @ a2], axis=1).astype(np.float16)


def _make_sel() -> np.ndarray:
    sel = np.zeros((NCH, NCH * P), dtype=np.float16)
    for c in range(NCH):
        sel[c, c * P:(c + 1) * P] = 1.0
    return sel


def _pack_adj(adj_slice: np.ndarray):
    """int32 [bpc, N, N] (adj[i,j]) -> fp8 [bpc, P, NCH, N] with
    packed[g, p, c, i] = adj[g, i, c*P+p]  (pre-transposed, partition-major)."""
    import ml_dtypes
    a = adj_slice.transpose(0, 2, 1)                   # [g, j, i]
    a = a.reshape(a.shape[0], NCH, P, N)               # [g, c, p, i]
    a = a.transpose(0, 2, 1, 3)                        # [g, p, c, i]
    return np.ascontiguousarray(a).astype(ml_dtypes.float8_e4m3)


def kernel(atoms_vector: np.ndarray, adjacency: np.ndarray, W: np.ndarray,
           a: np.ndarray) -> np.ndarray:
    from concourse.bass_utils import run_bass_kernel_spmd

    B = atoms_vector.shape[0]
    bpc = B // N_CORES
    wext = _make_wext(W, a)
    sel = _make_sel()

    nc = _get_nc(bpc)
    in_maps = []
    for i in range(N_CORES):
        sl = slice(i * bpc, (i + 1) * bpc)
        in_maps.append({
            "atomsT": np.ascontiguousarray(
                atoms_vector[sl].transpose(0, 2, 1)).astype(np.float16),
            "adj": _pack_adj(adjacency[sl]),
            "wext": wext,
            "selmat": sel,
        })
    res = run_bass_kernel_spmd(nc, in_maps, list(range(N_CORES)))
    return np.concatenate([res.results[i]["out"] for i in range(N_CORES)],
                      axis=0).astype(np.float32)


# revision 8
# speedup vs baseline: 1.3541x; 1.3541x over previous
"""GAT layer kernel for Trainium2 (8 NeuronCores, data-parallel over batch).

Reference computation (per graph b):
    Wh  = atoms @ W                      (N, FO)
    s1  = Wh @ a1 ; s2 = Wh @ a2         (N,)
    e   = leaky_relu(s1[:,None]+s2[None,:], 0.1)
    att = softmax(where(adj>0, e, -9e15), axis=1)
    out = elu(att @ Wh)

On-device formulation (no transcendental ever touches the NxN matrix):
    exp(leaky_relu(s)) = max(e^{s1_i + s2_j}, e^{0.1 (s1_i + s2_j)})
and because softmax row-normalizes, any per-row factor cancels; multiply
rows by e^{-0.1 s1_i} and factor out the column term v_j = e^{s2_j - 3}:
    B'_ij = v_j * max(w_i, u_j),  w = e^{0.9 s1} (clamped), u = e^{-0.9 s2}
    att_ij = adj_ij B'_ij / sum_j adj_ij B'_ij
The clamps (60000, fp16-range guards) bind only when both sides exceed
them, which the score statistics make vanishingly rare.  v_j folds into
the matmul rhs [v*Wh | v], so the denominator still comes free as the
65th column, and the whole NxN score+mask pass collapses to ONE fused
DVE scalar_tensor_tensor per j-chunk:  pm[j,:] = (w_bcast max u_j) * adjT.

HBM-traffic-lean layout (the target_regime is memory):
  * adjacency ships host-pre-transposed (adj[g,j,i]=adjacency[g,i,j]),
    host-packed to fp8e4 (0/1 exact, 1/4 of the int32 bytes and 1/2 of an
    fp16 shipment), and host-rearranged partition-major [g, p, c, i] so
    per-partition DRAM runs are fat.  Two gpsimd (SWDGE) cast-DMAs per
    graph upcast fp8 -> fp16 in flight, so the DVE mask pass runs on
    2-byte operands at full DVE speed (stt measures ~2x mode on HW even
    though the cost model charges 1x).
  * atoms ship host-transposed+fp16 [g, F_IN, N]: the on-device PE
    block-transposes and their PSUM staging disappear entirely.
  * output ships fp16 and is upcast on the host (elu output is O(1)).
gpsimd gotchas baked in here: no scalar_tensor_tensor, no tensor_tensor
max, no PSUM access, and tensor_scalar with AP (pointer) scalars is
pathologically slow — gpsimd only runs the SWDGE DMAs and one
immediate-scalar op; everything pointer-scalar lives on DVE/ACT.
Attention matmuls accumulate h rows for all 8 i-chunks across the jc
loop in two PSUM banks per graph, ic-major so each PSUM region's
accumulation group is contiguous (interleaved groups within a bank
lose all but the last-started group's accumulation on HW).  The softmax
divide folds into ACT Copy activations with a reciprocal scale pointer.
"""

import numpy as np
from contextlib import ExitStack

import concourse.bass as bass
import concourse.tile as tile
import concourse.mybir as mybir
from concourse.masks import make_identity

dt = mybir.dt
Alu = mybir.AluOpType
Act = mybir.ActivationFunctionType

N = 1024          # nodes per graph
F_IN = 128        # input features
FO = 64           # output features
P = 128           # partitions
NCH = N // P      # 8 node chunks
N_CORES = 8
B_FULL = 64
M_SHIFT = 6.0     # exponent recentering for v = exp(s2 - 3)

# engine-split knobs (tuned via timeline sim)
STT_DVE = 8       # j-chunks fused on DVE; the rest Pool-em + DVE-mask
EM1_POOL = True   # elu exp-1/min piece on gpsimd instead of DVE
HDIV_POOL = False  # gpsimd cannot access PSUM; ACT does the divide


def build_gat(bpc: int, reps: int = 1) -> bass.Bass:
    """Emit the bass program for one core processing `bpc` graphs."""
    nc = bass.Bass()
    atomsT = nc.declare_dram_parameter("atomsT", [bpc, F_IN, N], dt.float16,
                                       isOutput=False)
    # adjacency values are 0/1 — shipped host-repacked as fp8e4 (exact, 1/4
    # the int32 HBM traffic), host-PRE-TRANSPOSED (adj[g,j,i]=adjacency[g,i,j])
    # and partition-major so one SWDGE cast-DMA per graph lands fp16 in SBUF.
    adj = nc.declare_dram_parameter("adj", [bpc, P, NCH, N], dt.float8e4,
                                    isOutput=False)
    wext = nc.declare_dram_parameter("wext", [F_IN, FO + 2], dt.float16,
                                     isOutput=False)
    selmat = nc.declare_dram_parameter("selmat", [NCH, NCH * P], dt.float16,
                                       isOutput=False)
    out = nc.declare_dram_parameter("out", [bpc, N, FO], dt.float16, isOutput=True)

    with tile.TileContext(nc) as tc, ExitStack() as ctx:
        consts = ctx.enter_context(tc.tile_pool(name="consts", bufs=1))
        # PSUM budget (8 banks x 2KB): h 4 + t 2 + mm 2
        psum = ctx.enter_context(tc.tile_pool(name="psum", bufs=2, space="PSUM"))
        gbuf = ctx.enter_context(tc.tile_pool(name="gbuf", bufs=2))
        cbuf = ctx.enter_context(tc.tile_pool(name="cbuf", bufs=4))
        fbuf = ctx.enter_context(tc.tile_pool(name="fbuf", bufs=2))

        ident_b = consts.tile([P, P], dt.float16, tag="idb")
        make_identity(nc, ident_b)
        wext_sb = consts.tile([P, FO + 2], dt.float16, tag="wext")
        nc.sync.dma_start(out=wext_sb, in_=wext[:, :])
        bias_mh = consts.tile([P, 1], dt.float32, tag="bmh")
        nc.vector.memset(bias_mh, -M_SHIFT / 2)
        bias_z = consts.tile([P, 1], dt.float32, tag="bz")
        nc.vector.memset(bias_z, 0.0)
        # sel[:, c*P:(c+1)*P] is all-ones in row c: K=8 matmul with it as
        # stationary broadcasts row c of an [8, 128] tile to all partitions.
        sel_sb = consts.tile([NCH, NCH * P], dt.float16, tag="sel")
        nc.sync.dma_start(out=sel_sb, in_=selmat[:, :])

        def precompute(g):
            # ---------------- per-graph precompute (small) ----------------
            atT_sb = gbuf.tile([P, N], dt.float16, tag="atT", name=f"atT_{g}")
            nc.sync.dma_start(out=atT_sb, in_=atomsT[g])

            # [Wh | s1 | s2] = atoms_chunk @ [W | Wa1 | Wa2]; Wh and the
            # ones column get scaled by v_j = exp(s2_j - 3) (the column
            # factor of B'_ij = v_j * max(w_i, u_j)).
            whones = gbuf.tile([P, NCH, FO + 1], dt.float16, tag="whones",
                               name=f"whones_{g}")
            s12 = gbuf.tile([P, NCH, 2], dt.float32, tag="s12", name=f"s12_{g}")
            vcols = gbuf.tile([P, NCH], dt.float32, tag="vcols", name=f"vcols_{g}")
            for h in range(2):
                whc_ps = psum.tile([P, 4, FO + 2], dt.float32, tag="t",
                                   name=f"whc_ps_{g}_{h}")
                for cc in range(4):
                    c = h * 4 + cc
                    nc.tensor.matmul(whc_ps[:, cc, :],
                                     lhsT=atT_sb[:, c * P:(c + 1) * P],
                                     rhs=wext_sb, start=True, stop=True)
                nc.vector.tensor_copy(out=s12[:, h * 4:(h + 1) * 4, :],
                                      in_=whc_ps[:, :, FO:FO + 2])
                nc.scalar.activation(vcols[:, h * 4:(h + 1) * 4],
                                     s12[:, h * 4:(h + 1) * 4, 1], Act.Exp,
                                     bias=bias_mh, scale=1.0)
                for cc in range(4):
                    c = h * 4 + cc
                    nc.scalar.mul(whones[:, c, 0:FO], whc_ps[:, cc, 0:FO],
                                  vcols[:, c:c + 1])
            nc.vector.tensor_copy(out=whones[:, :, FO], in_=vcols)

            # w_i = min(exp(0.9 s1), 60000) broadcast; u_j = min(exp(-0.9 s2), 60000)
            rraw = gbuf.tile([P, NCH], dt.float32, tag="rraw", name=f"rraw_{g}")
            nc.scalar.activation(rraw, s12[:, :, 0], Act.Exp, bias=bias_z, scale=0.9)
            rcols16 = gbuf.tile([P, NCH], dt.float16, tag="rcols16",
                                name=f"rcols16_{g}")
            nc.vector.tensor_scalar(rcols16, rraw, 60000.0, None, Alu.min)
            uraw = gbuf.tile([P, NCH], dt.float32, tag="uraw", name=f"uraw_{g}")
            nc.scalar.activation(uraw, s12[:, :, 1], Act.Exp, bias=bias_z, scale=-0.9)
            qcols = gbuf.tile([P, NCH], dt.float32, tag="qcols", name=f"qcols_{g}")
            nc.vector.tensor_scalar(qcols, uraw, 60000.0, None, Alu.min)

            # broadcast w across partitions: rb[p, i] = w_i
            rT_ps = psum.tile([NCH, P], dt.float16, tag="t", name=f"rT_ps_{g}")
            nc.tensor.transpose(rT_ps, rcols16, ident_b)
            rT_sb = gbuf.tile([NCH, P], dt.float16, tag="rT", name=f"rT_{g}")
            nc.vector.tensor_copy(out=rT_sb, in_=rT_ps)
            rb_ps = psum.tile([P, N], dt.float32, tag="mm", bufs=1, name=f"rb_ps_{g}")
            for c in range(NCH):
                nc.tensor.matmul(rb_ps[:, c * P:(c + 1) * P],
                                 lhsT=sel_sb[:, c * P:(c + 1) * P],
                                 rhs=rT_sb, start=True, stop=True)
            rb = gbuf.tile([P, N], dt.float16, tag="rb", name=f"rb_{g}")
            nc.scalar.copy(out=rb, in_=rb_ps)

            adjf = [cbuf.tile([P, 4, N], dt.float16, tag="adjf", bufs=4,
                              name=f"adjf_{g}_{h}") for h in range(2)]
            h_ps = [psum.tile([P, 4, P], dt.float32, tag="h", bufs=4,
                              name=f"h_ps_{g}_{t}") for t in range(2)]
            res_g = gbuf.tile([P, NCH, FO], dt.float16, tag="res", name=f"res_{g}")
            return dict(adjf=adjf, whones=whones, vcols=vcols, qcols=qcols, rb=rb,
                        h_ps=h_ps, res=res_g, pms=[], ems=[])

        def adj_load(g, st):
            # SWDGE cast-DMAs: fp8 in DRAM -> fp16 in SBUF.  Two halves in
            # separate tiles so the first stt rows start after half the
            # transfer (partition-major host layout = fat descriptors).
            for h in range(2):
                nc.gpsimd.dma_start(out=st["adjf"][h], in_=adj[g][:, 4 * h:4 * h + 4, :])

        def emit_ems(g, st):
            # all j-chunks run as fused DVE scalar_tensor_tensor in pm_steps;
            # gpsimd instead owns the SWDGE cast-DMA and finalize pieces.
            if STT_DVE < NCH:
                em = cbuf.tile([P, NCH - STT_DVE, N], dt.float16, tag="em",
                               bufs=4, name=f"em_{g}")
                st["em"] = em
                for k, jc in enumerate(range(STT_DVE, NCH)):
                    nc.gpsimd.tensor_scalar(em[:, k, :], st["rb"],
                                            st["qcols"][:, jc:jc + 1], None,
                                            Alu.max)

        def pm_steps(g, st):
            # fused masked P^T rows: pm[j,:] = max(w, u_j) * adjT[j,:].
            # jc < STT_DVE: single DVE scalar_tensor_tensor; the rest mask
            # Pool-built em rows with a DVE tensor_tensor.
            for h in range(2):
                pm = cbuf.tile([P, 4, N], dt.float16, tag="pm", bufs=4,
                               name=f"pm_{g}_{h}")
                st["pms"].append(pm)
            for jc in range(STT_DVE):
                nc.vector.scalar_tensor_tensor(st["pms"][jc // 4][:, jc % 4, :],
                                               st["rb"], st["qcols"][:, jc:jc + 1],
                                               st["adjf"][jc // 4][:, jc % 4, :],
                                               Alu.max, Alu.mult)
            for k, jc in enumerate(range(STT_DVE, NCH)):
                nc.vector.tensor_tensor(st["pms"][jc // 4][:, jc % 4, :],
                                        st["em"][:, k, :],
                                        st["adjf"][jc // 4][:, jc % 4, :], Alu.mult)

        def attn_mm(g, st, ic0=0, ic1=NCH):
            # h[i, 0:64] + denom col.  ic-major: each PSUM region's 8-matmul
            # accumulation group is contiguous — interleaved groups within a
            # bank lose all but the last-started group's accumulation on HW.
            for ic in range(ic0, ic1):
                for jc in range(NCH):
                    nc.tensor.matmul(st["h_ps"][ic // 4][:, ic % 4, 0:FO + 1],
                                     lhsT=st["pms"][jc // 4][:, jc % 4,
                                                            ic * P:(ic + 1) * P],
                                     rhs=st["whones"][:, jc, :],
                                     start=(jc == 0), stop=(jc == NCH - 1))

        def finalize(g, st):
            h_ps, res_g = st["h_ps"], st["res"]
            rec = fbuf.tile([P, NCH], dt.float32, tag="rec", name=f"rec_{g}")
            for t in range(2):
                nc.vector.reciprocal(rec[:, t * 4:(t + 1) * 4],
                                     h_ps[t][:, :, FO:FO + 1])
            hdiv = fbuf.tile([P, NCH, FO], dt.float32, tag="hdiv", name=f"hdiv_{g}")
            for c in range(NCH):
                eng = nc.gpsimd if HDIV_POOL else nc.scalar
                if eng is nc.scalar:
                    nc.scalar.mul(hdiv[:, c, :], h_ps[c // 4][:, c % 4, 0:FO],
                                  rec[:, c:c + 1])
                else:
                    eng.tensor_scalar(hdiv[:, c, :], h_ps[c // 4][:, c % 4, 0:FO],
                                      rec[:, c:c + 1], None, Alu.mult)
            hexp = fbuf.tile([P, NCH, FO], dt.float32, tag="hexp", name=f"hexp_{g}")
            nc.scalar.activation(hexp, hdiv, Act.Exp, bias=bias_z)
            em1 = fbuf.tile([P, NCH, FO], dt.float32, tag="em1", name=f"em1_{g}")
            eng = nc.gpsimd if EM1_POOL else nc.vector
            eng.tensor_scalar(em1, hexp, -1.0, 0.0, Alu.add, Alu.min)
            nc.vector.tensor_tensor(res_g, hdiv, em1, Alu.max)
            nc.scalar.dma_start(out=out[g].rearrange("(c p) f -> p c f", p=P),
                                in_=res_g)

        def start_graph(g):
            st = precompute(g)
            adj_load(g, st)
            return st

        for _rep in range(reps):
            states = {0: start_graph(0)}
            emit_ems(0, states[0])
            done = {}
            for g in range(bpc):
                if g + 1 < bpc:
                    states[g + 1] = start_graph(g + 1)
                st = states.pop(g)
                pm_steps(g, st)
                attn_mm(g, st)
                if g >= 1:
                    finalize(g - 1, done.pop(g - 1))
                done[g] = st
                if g + 1 < bpc:
                    emit_ems(g + 1, states[g + 1])
            finalize(bpc - 1, done.pop(bpc - 1))

    # HW allows at most one sync-wait per Matmult/Ldweights; Tile can emit
    # more.  Run the bacc lowering passes that move extra waits onto
    # ldweights / standalone event-semaphore instructions.
    import bass_rust as _br
    _br.move_matmul_waits_to_ldweights(nc.m)
    _br.generate_event_semaphores(nc)
    return nc


_NC_CACHE: dict[int, bass.Bass] = {}


def _get_nc(bpc: int) -> bass.Bass:
    if bpc not in _NC_CACHE:
        _NC_CACHE[bpc] = build_gat(bpc)
    return _NC_CACHE[bpc]


def _make_wext(W: np.ndarray, a: np.ndarray) -> np.ndarray:
    a1 = a[:FO, :]
    a2 = a[FO:, :]
    return np.concatenate([W, W @ a1, W @ a2], axis=1).astype(np.float16)


def _make_sel() -> np.ndarray:
    sel = np.zeros((NCH, NCH * P), dtype=np.float16)
    for c in range(NCH):
        sel[c, c * P:(c + 1) * P] = 1.0
    return sel


def _pack_adj(adj_slice: np.ndarray):
    """int32 [bpc, N, N] (adj[i,j]) -> fp8 [bpc, P, NCH, N] with
    packed[g, p, c, i] = adj[g, i, c*P+p]  (pre-transposed, partition-major)."""
    import ml_dtypes
    a = adj_slice.transpose(0, 2, 1)                   # [g, j, i]
    a = a.reshape(a.shape[0], NCH, P, N)               # [g, c, p, i]
    a = a.transpose(0, 2, 1, 3)                        # [g, p, c, i]
    return np.ascontiguousarray(a).astype(ml_dtypes.float8_e4m3)


def kernel(atoms_vector: np.ndarray, adjacency: np.ndarray, W: np.ndarray,
           a: np.ndarray) -> np.ndarray:
    from concourse.bass_utils import run_bass_kernel_spmd

    B = atoms_vector.shape[0]
    bpc = B // N_CORES
    wext = _make_wext(W, a)
    sel = _make_sel()

    nc = _get_nc(bpc)
    in_maps = []
    for i in range(N_CORES):
        sl = slice(i * bpc, (i + 1) * bpc)
        in_maps.append({
            "atomsT": np.ascontiguousarray(
                atoms_vector[sl].transpose(0, 2, 1)).astype(np.float16),
            "adj": _pack_adj(adjacency[sl]),
            "wext": wext,
            "selmat": sel,
        })
    res = run_bass_kernel_spmd(nc, in_maps, list(range(N_CORES)))
    return np.concatenate([res.results[i]["out"] for i in range(N_CORES)],
                      axis=0).astype(np.float32)



# revision 9
# speedup vs baseline: 1.3745x; 1.0151x over previous
"""GAT layer kernel for Trainium2 (8 NeuronCores, data-parallel over batch).

Reference computation (per graph b):
    Wh  = atoms @ W                      (N, FO)
    s1  = Wh @ a1 ; s2 = Wh @ a2         (N,)
    e   = leaky_relu(s1[:,None]+s2[None,:], 0.1)
    att = softmax(where(adj>0, e, -9e15), axis=1)
    out = elu(att @ Wh)

On-device formulation (no transcendental ever touches the NxN matrix):
    exp(leaky_relu(s)) = max(e^{s1_i + s2_j}, e^{0.1 (s1_i + s2_j)})
and because softmax row-normalizes, any per-row factor cancels; multiply
rows by e^{-0.1 s1_i} and factor out the column term v_j = e^{s2_j - 3}:
    B'_ij = v_j * max(w_i, u_j),  w = e^{0.9 s1} (clamped), u = e^{-0.9 s2}
    att_ij = adj_ij B'_ij / sum_j adj_ij B'_ij
The clamps (60000, fp16-range guards) bind only when both sides exceed
them, which the score statistics make vanishingly rare.  v_j folds into
the matmul rhs [v*Wh | v], so the denominator still comes free as the
65th column, and the whole NxN score+mask pass collapses to ONE fused
DVE scalar_tensor_tensor per j-chunk:  pm[j,:] = (w_bcast max u_j) * adjT.

HBM-traffic-lean layout (the target_regime is memory):
  * adjacency ships host-pre-transposed (adj[g,j,i]=adjacency[g,i,j]),
    host-packed to fp8e4 (0/1 exact, 1/4 of the int32 bytes and 1/2 of an
    fp16 shipment), and host-rearranged partition-major [g, p, c, i] so
    per-partition DRAM runs are fat.  Two gpsimd (SWDGE) cast-DMAs per
    graph upcast fp8 -> fp16 in flight, so the DVE mask pass runs on
    2-byte operands at full DVE speed (stt measures ~2x mode on HW even
    though the cost model charges 1x).
  * atoms ship host-transposed+fp16 [g, F_IN, N]: the on-device PE
    block-transposes and their PSUM staging disappear entirely.
  * output ships fp16 and is upcast on the host (elu output is O(1)).
gpsimd gotchas baked in here: no scalar_tensor_tensor, no tensor_tensor
max, no PSUM access, and tensor_scalar with AP (pointer) scalars is
pathologically slow — gpsimd only runs the SWDGE DMAs and one
immediate-scalar op; everything pointer-scalar lives on DVE/ACT.
Attention matmuls accumulate h rows for all 8 i-chunks across the jc
loop in two PSUM banks per graph, ic-major so each PSUM region's
accumulation group is contiguous (interleaved groups within a bank
lose all but the last-started group's accumulation on HW).  The softmax
divide folds into ACT Copy activations with a reciprocal scale pointer.
"""

import numpy as np
from contextlib import ExitStack

import concourse.bass as bass
import concourse.tile as tile
import concourse.mybir as mybir
from concourse.masks import make_identity

dt = mybir.dt
Alu = mybir.AluOpType
Act = mybir.ActivationFunctionType

N = 1024          # nodes per graph
F_IN = 128        # input features
FO = 64           # output features
P = 128           # partitions
NCH = N // P      # 8 node chunks
N_CORES = 8
B_FULL = 64
M_SHIFT = 6.0     # exponent recentering for v = exp(s2 - 3)

# engine-split knobs (tuned via timeline sim)
STT_DVE = 8       # j-chunks fused on DVE; the rest Pool-em + DVE-mask
EM1_POOL = True   # elu exp-1/min piece on gpsimd instead of DVE
HDIV_POOL = False  # gpsimd cannot access PSUM; ACT does the divide


def build_gat(bpc: int, reps: int = 1) -> bass.Bass:
    """Emit the bass program for one core processing `bpc` graphs."""
    nc = bass.Bass()
    atomsT = nc.declare_dram_parameter("atomsT", [bpc, F_IN, N], dt.float16,
                                       isOutput=False)
    # adjacency values are 0/1 — shipped host-repacked as fp8e4 (exact, 1/4
    # the int32 HBM traffic), host-PRE-TRANSPOSED (adj[g,j,i]=adjacency[g,i,j])
    # and partition-major so one SWDGE cast-DMA per graph lands fp16 in SBUF.
    adj = nc.declare_dram_parameter("adj", [bpc, P, NCH, N], dt.float8e4,
                                    isOutput=False)
    wext = nc.declare_dram_parameter("wext", [F_IN, FO + 2], dt.float16,
                                     isOutput=False)
    selmat = nc.declare_dram_parameter("selmat", [NCH, NCH * P], dt.float16,
                                       isOutput=False)
    out = nc.declare_dram_parameter("out", [bpc, N, FO], dt.float16, isOutput=True)

    with tile.TileContext(nc) as tc, ExitStack() as ctx:
        consts = ctx.enter_context(tc.tile_pool(name="consts", bufs=1))
        # PSUM budget (8 banks x 2KB): h 4 + t 2 + mm 2
        psum = ctx.enter_context(tc.tile_pool(name="psum", bufs=2, space="PSUM"))
        gbuf = ctx.enter_context(tc.tile_pool(name="gbuf", bufs=2))
        cbuf = ctx.enter_context(tc.tile_pool(name="cbuf", bufs=4))
        fbuf = ctx.enter_context(tc.tile_pool(name="fbuf", bufs=2))

        ident_b = consts.tile([P, P], dt.float16, tag="idb")
        make_identity(nc, ident_b)
        wext_sb = consts.tile([P, FO + 2], dt.float16, tag="wext")
        nc.sync.dma_start(out=wext_sb, in_=wext[:, :])
        bias_mh = consts.tile([P, 1], dt.float32, tag="bmh")
        nc.vector.memset(bias_mh, -M_SHIFT / 2)
        bias_z = consts.tile([P, 1], dt.float32, tag="bz")
        nc.vector.memset(bias_z, 0.0)
        # sel[:, c*P:(c+1)*P] is all-ones in row c: K=8 matmul with it as
        # stationary broadcasts row c of an [8, 128] tile to all partitions.
        sel_sb = consts.tile([NCH, NCH * P], dt.float16, tag="sel")
        nc.sync.dma_start(out=sel_sb, in_=selmat[:, :])

        def precompute(g):
            # ---------------- per-graph precompute (small) ----------------
            atT_sb = gbuf.tile([P, N], dt.float16, tag="atT", name=f"atT_{g}")
            nc.sync.dma_start(out=atT_sb, in_=atomsT[g])

            # [Wh | s1 | s2] = atoms_chunk @ [W | Wa1 | Wa2]; Wh and the
            # ones column get scaled by v_j = exp(s2_j - 3) (the column
            # factor of B'_ij = v_j * max(w_i, u_j)).
            whones = gbuf.tile([P, NCH, FO + 1], dt.float16, tag="whones",
                               name=f"whones_{g}")
            s12 = gbuf.tile([P, NCH, 2], dt.float32, tag="s12", name=f"s12_{g}")
            vcols = gbuf.tile([P, NCH], dt.float32, tag="vcols", name=f"vcols_{g}")
            for h in range(2):
                whc_ps = psum.tile([P, 4, FO + 2], dt.float32, tag="t",
                                   name=f"whc_ps_{g}_{h}")
                for cc in range(4):
                    c = h * 4 + cc
                    nc.tensor.matmul(whc_ps[:, cc, :],
                                     lhsT=atT_sb[:, c * P:(c + 1) * P],
                                     rhs=wext_sb, start=True, stop=True)
                nc.vector.tensor_copy(out=s12[:, h * 4:(h + 1) * 4, :],
                                      in_=whc_ps[:, :, FO:FO + 2])
                nc.scalar.activation(vcols[:, h * 4:(h + 1) * 4],
                                     s12[:, h * 4:(h + 1) * 4, 1], Act.Exp,
                                     bias=bias_mh, scale=1.0)
                for cc in range(4):
                    c = h * 4 + cc
                    nc.scalar.mul(whones[:, c, 0:FO], whc_ps[:, cc, 0:FO],
                                  vcols[:, c:c + 1])
            nc.vector.tensor_copy(out=whones[:, :, FO], in_=vcols)

            # w_i = min(exp(0.9 s1), 60000) broadcast; u_j = min(exp(-0.9 s2), 60000)
            rraw = gbuf.tile([P, NCH], dt.float32, tag="rraw", name=f"rraw_{g}")
            nc.scalar.activation(rraw, s12[:, :, 0], Act.Exp, bias=bias_z, scale=0.9)
            rcols16 = gbuf.tile([P, NCH], dt.float16, tag="rcols16",
                                name=f"rcols16_{g}")
            nc.vector.tensor_scalar(rcols16, rraw, 60000.0, None, Alu.min)
            uraw = gbuf.tile([P, NCH], dt.float32, tag="uraw", name=f"uraw_{g}")
            nc.scalar.activation(uraw, s12[:, :, 1], Act.Exp, bias=bias_z, scale=-0.9)
            qcols = gbuf.tile([P, NCH], dt.float32, tag="qcols", name=f"qcols_{g}")
            nc.vector.tensor_scalar(qcols, uraw, 60000.0, None, Alu.min)

            # broadcast w across partitions: rb[p, i] = w_i
            rT_ps = psum.tile([NCH, P], dt.float16, tag="t", name=f"rT_ps_{g}")
            nc.tensor.transpose(rT_ps, rcols16, ident_b)
            rT_sb = gbuf.tile([NCH, P], dt.float16, tag="rT", name=f"rT_{g}")
            nc.vector.tensor_copy(out=rT_sb, in_=rT_ps)
            rb_ps = psum.tile([P, N], dt.float32, tag="mm", bufs=1, name=f"rb_ps_{g}")
            for c in range(NCH):
                nc.tensor.matmul(rb_ps[:, c * P:(c + 1) * P],
                                 lhsT=sel_sb[:, c * P:(c + 1) * P],
                                 rhs=rT_sb, start=True, stop=True)
            rb = gbuf.tile([P, N], dt.float16, tag="rb", name=f"rb_{g}")
            nc.scalar.copy(out=rb, in_=rb_ps)

            adjf = [cbuf.tile([P, 4, N], dt.float16, tag="adjf", bufs=4,
                              name=f"adjf_{g}_{h}") for h in range(2)]
            h_ps = [psum.tile([P, 4, P], dt.float32, tag="h", bufs=4,
                              name=f"h_ps_{g}_{t}") for t in range(2)]
            res_g = gbuf.tile([P, NCH, FO], dt.float16, tag="res", name=f"res_{g}")
            return dict(adjf=adjf, whones=whones, vcols=vcols, qcols=qcols, rb=rb,
                        h_ps=h_ps, res=res_g, pms=[], ems=[])

        def adj_load(g, st):
            # SWDGE cast-DMAs: fp8 in DRAM -> fp16 in SBUF.  Two halves in
            # separate tiles so the first stt rows start after half the
            # transfer (partition-major host layout = fat descriptors).
            for h in range(2):
                nc.gpsimd.dma_start(out=st["adjf"][h], in_=adj[g][:, 4 * h:4 * h + 4, :])

        def emit_ems(g, st):
            # all j-chunks run as fused DVE scalar_tensor_tensor in pm_steps;
            # gpsimd instead owns the SWDGE cast-DMA and finalize pieces.
            if STT_DVE < NCH:
                em = cbuf.tile([P, NCH - STT_DVE, N], dt.float16, tag="em",
                               bufs=4, name=f"em_{g}")
                st["em"] = em
                for k, jc in enumerate(range(STT_DVE, NCH)):
                    nc.gpsimd.tensor_scalar(em[:, k, :], st["rb"],
                                            st["qcols"][:, jc:jc + 1], None,
                                            Alu.max)

        def pm_steps(g, st):
            # fused masked P^T rows: pm[j,:] = max(w, u_j) * adjT[j,:].
            # jc < STT_DVE: single DVE scalar_tensor_tensor; the rest mask
            # Pool-built em rows with a DVE tensor_tensor.
            for h in range(2):
                pm = cbuf.tile([P, 4, N], dt.float16, tag="pm", bufs=4,
                               name=f"pm_{g}_{h}")
                st["pms"].append(pm)
            for jc in range(STT_DVE):
                nc.vector.scalar_tensor_tensor(st["pms"][jc // 4][:, jc % 4, :],
                                               st["rb"], st["qcols"][:, jc:jc + 1],
                                               st["adjf"][jc // 4][:, jc % 4, :],
                                               Alu.max, Alu.mult)
            for k, jc in enumerate(range(STT_DVE, NCH)):
                nc.vector.tensor_tensor(st["pms"][jc // 4][:, jc % 4, :],
                                        st["em"][:, k, :],
                                        st["adjf"][jc // 4][:, jc % 4, :], Alu.mult)

        def attn_mm(g, st, ic0=0, ic1=NCH):
            # h[i, 0:64] + denom col.  ic-major: each PSUM region's 8-matmul
            # accumulation group is contiguous — interleaved groups within a
            # bank lose all but the last-started group's accumulation on HW.
            for ic in range(ic0, ic1):
                for jc in range(NCH):
                    nc.tensor.matmul(st["h_ps"][ic // 4][:, ic % 4, 0:FO + 1],
                                     lhsT=st["pms"][jc // 4][:, jc % 4,
                                                            ic * P:(ic + 1) * P],
                                     rhs=st["whones"][:, jc, :],
                                     start=(jc == 0), stop=(jc == NCH - 1))

        def finalize(g, st):
            h_ps, res_g = st["h_ps"], st["res"]
            rec = fbuf.tile([P, NCH], dt.float32, tag="rec", name=f"rec_{g}")
            for t in range(2):
                nc.vector.reciprocal(rec[:, t * 4:(t + 1) * 4],
                                     h_ps[t][:, :, FO:FO + 1])
            hdiv = fbuf.tile([P, NCH, FO], dt.float16, tag="hdiv", name=f"hdiv_{g}")
            for c in range(NCH):
                eng = nc.gpsimd if HDIV_POOL else nc.scalar
                if eng is nc.scalar:
                    nc.scalar.mul(hdiv[:, c, :], h_ps[c // 4][:, c % 4, 0:FO],
                                  rec[:, c:c + 1])
                else:
                    eng.tensor_scalar(hdiv[:, c, :], h_ps[c // 4][:, c % 4, 0:FO],
                                      rec[:, c:c + 1], None, Alu.mult)
            hexp = fbuf.tile([P, NCH, FO], dt.float16, tag="hexp", name=f"hexp_{g}")
            nc.scalar.activation(hexp, hdiv, Act.Exp, bias=bias_z)
            em1 = fbuf.tile([P, NCH, FO], dt.float16, tag="em1", name=f"em1_{g}")
            eng = nc.gpsimd if EM1_POOL else nc.vector
            eng.tensor_scalar(em1, hexp, -1.0, 0.0, Alu.add, Alu.min)
            nc.vector.tensor_tensor(res_g, hdiv, em1, Alu.max)
            nc.scalar.dma_start(out=out[g].rearrange("(c p) f -> p c f", p=P),
                                in_=res_g)

        def start_graph(g):
            st = precompute(g)
            adj_load(g, st)
            return st

        for _rep in range(reps):
            states = {0: start_graph(0)}
            emit_ems(0, states[0])
            done = {}
            for g in range(bpc):
                if g + 1 < bpc:
                    states[g + 1] = start_graph(g + 1)
                st = states.pop(g)
                pm_steps(g, st)
                attn_mm(g, st)
                if g >= 1:
                    finalize(g - 1, done.pop(g - 1))
                done[g] = st
                if g + 1 < bpc:
                    emit_ems(g + 1, states[g + 1])
            finalize(bpc - 1, done.pop(bpc - 1))

    # HW allows at most one sync-wait per Matmult/Ldweights; Tile can emit
    # more.  Run the bacc lowering passes that move extra waits onto
    # ldweights / standalone event-semaphore instructions.
    import bass_rust as _br
    _br.move_matmul_waits_to_ldweights(nc.m)
    _br.generate_event_semaphores(nc)
    return nc


_NC_CACHE: dict[int, bass.Bass] = {}


def _get_nc(bpc: int) -> bass.Bass:
    if bpc not in _NC_CACHE:
        _NC_CACHE[bpc] = build_gat(bpc)
    return _NC_CACHE[bpc]


def _make_wext(W: np.ndarray, a: np.ndarray) -> np.ndarray:
    a1 = a[:FO, :]
    a2 = a[FO:, :]
    return np.concatenate([W, W @ a1, W @ a2], axis=1).astype(np.float16)


def _make_sel() -> np.ndarray:
    sel = np.zeros((NCH, NCH * P), dtype=np.float16)
    for c in range(NCH):
        sel[c, c * P:(c + 1) * P] = 1.0
    return sel


def _pack_adj(adj_slice: np.ndarray):
    """int32 [bpc, N, N] (adj[i,j]) -> fp8 [bpc, P, NCH, N] with
    packed[g, p, c, i] = adj[g, i, c*P+p]  (pre-transposed, partition-major)."""
    import ml_dtypes
    a = adj_slice.transpose(0, 2, 1)                   # [g, j, i]
    a = a.reshape(a.shape[0], NCH, P, N)               # [g, c, p, i]
    a = a.transpose(0, 2, 1, 3)                        # [g, p, c, i]
    return np.ascontiguousarray(a).astype(ml_dtypes.float8_e4m3)


def kernel(atoms_vector: np.ndarray, adjacency: np.ndarray, W: np.ndarray,
           a: np.ndarray) -> np.ndarray:
    from concourse.bass_utils import run_bass_kernel_spmd

    B = atoms_vector.shape[0]
    bpc = B // N_CORES
    wext = _make_wext(W, a)
    sel = _make_sel()

    nc = _get_nc(bpc)
    in_maps = []
    for i in range(N_CORES):
        sl = slice(i * bpc, (i + 1) * bpc)
        in_maps.append({
            "atomsT": np.ascontiguousarray(
                atoms_vector[sl].transpose(0, 2, 1)).astype(np.float16),
            "adj": _pack_adj(adjacency[sl]),
            "wext": wext,
            "selmat": sel,
        })
    res = run_bass_kernel_spmd(nc, in_maps, list(range(N_CORES)))
    return np.concatenate([res.results[i]["out"] for i in range(N_CORES)],
                      axis=0).astype(np.float32)

